# revision 1
# baseline (speedup 1.0000x reference)
"""Trainium2 Bass kernel for cosine-sim multi-head attention.

Model (per batch element):
    xn  = l2norm(x) * g * sqrt(D)
    qkv = xn @ w_qkv ; split q,k,v ; heads of 64
    q   = l2norm(q) * q_scale ; k = l2norm(k) * k_scale
    out = softmax(8 * q k^T) v ; merge heads ; @ w_out

Sharding: data-parallel over batch (B=8) across the 8 NeuronCores.  Each
core runs an identical single-core program on its own batch element; there
are no collectives.

Key algebraic facts used:
  - the RMSNorm row scale cancels inside l2norm(q), l2norm(k); it only
    affects v.  So q,k are computed from *raw* x, and the row scale
    (sqrt(D)/||x_n||) is applied to v only (per-partition multiply).
  - g folds into the rows of w_qkv.
  - q_scale*k_scale folds into the k side of the QK^T product (per
    partition in the k^T layout).
  - scores are bounded: |S| <= 8 (l2-normalised q,k; scale 8), so softmax
    needs no max-subtraction: exp(S) in [3e-4, 3e3], fp32-safe.
  - S is computed *transposed* (S^T[j,i] = k_j . q_i) so that exp(S^T) can
    feed the attn@V matmul directly as the moving operand, with
    stationary [v_h | 1]: the appended ones column makes PSUM row 64 the
    softmax denominator for free.
  - 1/||k|| is applied via the ACTIVATE per-partition scale (exp(in*s)),
    1/||q|| is broadcast into q̂ via a DMA round trip through DRAM.
"""

import os
import sys

import numpy as np

sys.path.insert(0, "/opt/trn_rl_repo")

N = 2048
D = 512
H = 8
DH = 64
P = 128
NT = N // P  # 16 token tiles
DC = D // P  # 4 contraction chunks
SCALE = 8.0
NCORES = 8


def build_attention(nc, out_ap, x_ap, g_ap, wqkv_ap, qs_ap, ks_ap, wout_ap, dbg=None):
    """Emit the full single-core attention program into `nc` (Tile)."""
    import concourse.mybir as mybir
    from concourse.masks import make_identity
    from concourse.tile import TileContext

    f32 = mybir.dt.float32
    bf16 = mybir.dt.bfloat16
    f32r = mybir.dt.float32r

    def R(ap):
        return ap.bitcast(f32r)
    AF = mybir.ActivationFunctionType
    OP = mybir.AluOpType
    AX = mybir.AxisListType

    with TileContext(nc) as tc:
        # ---------------- persistent pools (released at the end, LIFO) -----
        const = tc.alloc_tile_pool(name="const", bufs=1)
        qkp = tc.alloc_tile_pool(name="qk", bufs=1)
        vp = tc.alloc_tile_pool(name="v1", bufs=1)
        dscr = tc.alloc_tile_pool(name="dscr", bufs=1, space="DRAM")
        drec = tc.alloc_tile_pool(name="drec", bufs=16, space="DRAM")

        ident = const.tile([P, P], f32)
        make_identity(nc, ident)
        # block-diagonal ones: the per-head partition-sum matmul directly
        # yields the sum replicated across that head's 64 partitions
        ind2s = const.tile([P, P], f32)
        nc.vector.memset(ind2s, 0.0)
        nc.vector.memset(ind2s[0:DH, 0:DH], 1.0)
        nc.vector.memset(ind2s[DH:P, DH:P], 1.0)
        ind2 = const.tile([P, P], f32)
        nc.sync.dma_start(R(ind2), R(ind2s))

        # q_scale * k_scale, replicated to 128 partitions via DRAM bounce
        qs_sb = const.tile([DH, 1], f32)
        ks_sb = const.tile([DH, 1], f32)
        nc.sync.dma_start(qs_sb, qs_ap[:, None])
        nc.sync.dma_start(ks_sb, ks_ap[:, None])
        qsks64 = const.tile([DH, 1], f32)
        nc.vector.tensor_tensor(qsks64, qs_sb, ks_sb, OP.mult)
        qsks_d = dscr.tile([DH], f32, tag="qsks")
        nc.sync.dma_start(qsks_d[:, None], qsks64)
        qsks = const.tile([P, 1], f32)
        nc.sync.dma_start(qsks[0:DH, :], qsks_d[:, None])
        nc.sync.dma_start(qsks[DH:P, :], qsks_d[:, None])

        # persistent big tensors
        qhat = qkp.tile([P, H // 2, N], f32)  # q^T, chunk m holds heads 2m,2m+1
        ktil = qkp.tile([P, H // 2, N], f32)  # k^T * qsks
        V1 = vp.tile([P, NT, H * (DH + 1)], bf16)  # per head [v_h | 1]

        srow = const.tile([P, NT], f32)  # sqrt(D)/||x_n|| per token
        ss_x = const.tile([P, NT], f32)


        # ---------------- phases 1-3 -------------------------------------
        with tc.tile_pool(name="xt", bufs=1) as xtp, \
             tc.tile_pool(name="wq", bufs=1) as wqp:

            wq = wqp.tile([P, DC, 3 * D], f32)
            xT = xtp.tile([P, DC, N], f32)

            # phase 1: load x natural, transpose to xT, row norms.  The x
            # tiles gate the whole pipeline, so they are loaded before the
            # weights.
            with tc.tile_pool(name="xnat", bufs=1) as xnp, \
                 tc.tile_pool(name="st1", bufs=2) as st1, \
                 tc.tile_pool(name="ps_tr", bufs=4, space="PSUM") as ps_tr:
                x_nat = xnp.tile([P, NT, D], f32)
                for t in range(NT):
                    nc.sync.dma_start(
                        x_nat[:, t, :],
                        x_ap.rearrange("(t p) d -> p t d", p=P)[:, t, :],
                    )
                for c in range(DC):
                    nc.sync.dma_start(
                        R(wq[:, c, :]),
                        R(wqkv_ap.rearrange("(c p) q -> p c q", p=P)[:, c, :]),
                    )
                g_sb = st1.tile([P, DC], f32, tag="g")
                nc.sync.dma_start(g_sb, g_ap.rearrange("(c p) -> p c", p=P))
                for c in range(DC):
                    nc.vector.tensor_scalar_mul(R(wq[:, c, :]), wq[:, c, :], g_sb[:, c : c + 1])

                # row sum-of-squares via ACT Square+accum; srow = sqrt(D)/||x_n||
                for t in range(NT):
                    xsq = st1.tile([P, D], f32, tag="xsq")
                    nc.scalar.activation(
                        xsq, x_nat[:, t, :], AF.Square,
                        accum_out=ss_x[:, t : t + 1],
                    )
                sqx = st1.tile([P, NT], f32, tag="sqx")
                nc.scalar.activation(sqx, ss_x, AF.Sqrt, scale=1.0 / D)
                nc.vector.reciprocal(srow, sqx)

                for t in range(NT):
                    for c in range(DC):
                        pst = ps_tr.tile([P, P], f32, tag="tr")
                        nc.tensor.transpose(
                            pst, x_nat[:, t, c * P : (c + 1) * P], ident
                        )
                        nc.vector.tensor_copy(R(xT[:, c, t * P : (t + 1) * P]), pst)

            # phase 2: q^T / k^T chunks + norms (k chunks first so the
            # attention phase's dependencies resolve earlier; half-chunk
            # PSUM tiles double-buffer the pipeline)
            with tc.tile_pool(name="ps_qkt", bufs=2, space="PSUM") as ps_qkt, \
                 tc.tile_pool(name="ps_ss", bufs=2, space="PSUM") as ps_ss, \
                 tc.tile_pool(name="sqp", bufs=2) as sqp, \
                 tc.tile_pool(name="bc", bufs=2) as bcp, \
                 tc.tile_pool(name="st2", bufs=2) as st2:
                for m in (4, 5, 6, 7, 0, 1, 2, 3):
                    dest = qhat[:, m, :] if m < 4 else ktil[:, m - 4, :]
                    rnb = bcp.tile([P, N], f32, tag="rnb", name=f"rnb{m}")
                    for ihalf in range(2):
                        io = ihalf * 1024
                        ps = ps_qkt.tile([P, 1024], f32, tag="qkt", name=f"qkt{m}_{ihalf}")
                        for c in range(DC):
                            for i4 in range(2):
                                nc.tensor.matmul(
                                    ps[:, i4 * 512 : (i4 + 1) * 512],
                                    lhsT=R(wq[:, c, m * P : (m + 1) * P]),
                                    rhs=R(xT[:, c, io + i4 * 512 : io + (i4 + 1) * 512]),
                                    start=(c == 0),
                                    stop=(c == DC - 1),
                                )
                        nc.scalar.copy(R(dest[:, io : io + 1024]), ps)
                        sq = sqp.tile([P, 1024], f32, tag="sq", name=f"sq{m}_{ihalf}")
                        nc.scalar.activation(R(sq), ps, AF.Square)
                        # per-head sumsq, already replicated over each head's
                        # 64 partitions by the block-diagonal ones lhsT
                        ssb = ps_ss.tile([P, 1024], f32, tag="ss", name=f"ss{m}_{ihalf}")
                        for i4 in range(2):
                            nc.tensor.matmul(
                                ssb[:, i4 * 512 : (i4 + 1) * 512],
                                lhsT=R(ind2),
                                rhs=R(sq[:, i4 * 512 : (i4 + 1) * 512]),
                                start=True,
                                stop=True,
                            )
                        nc.scalar.activation(rnb[:, io : io + 1024], ssb, AF.Sqrt)
                    # 1/||.|| per (head, token), then fold into the chunk
                    nc.vector.reciprocal_approx_fast(out=rnb, in_=rnb)
                    nc.vector.tensor_tensor(R(dest), dest, rnb, OP.mult)
                    if m >= 4:
                        nc.vector.tensor_scalar_mul(R(dest), dest, qsks)

            # phase 3: V1 = [srow * v | 1] per head
            with tc.tile_pool(name="ps_v", bufs=2, space="PSUM") as ps_v:
                for h in range(H):
                    nc.vector.memset(
                        V1[:, :, h * (DH + 1) + DH : h * (DH + 1) + DH + 1], 1.0
                    )
                for t in range(NT):
                    psv = ps_v.tile([P, D], f32, tag="v")
                    for c in range(DC):
                        nc.tensor.matmul(
                            psv,
                            lhsT=R(xT[:, c, t * P : (t + 1) * P]),
                            rhs=R(wq[:, c, 2 * D : 3 * D]),
                            start=(c == 0),
                            stop=(c == DC - 1),
                        )
                    nc.scalar.activation(
                        V1[:, t, :].rearrange("p (h c) -> p h c", c=DH + 1)[
                            :, :, 0:DH
                        ],
                        psv.rearrange("p (h c) -> p h c", c=DH),
                        AF.Copy,
                        scale=srow[:, t : t + 1],
                    )

        if dbg is not None:
            nc.sync.dma_start(dbg["dbg_srow"], srow)
            nc.sync.dma_start(dbg["dbg_qhat"], qhat)
            nc.sync.dma_start(dbg["dbg_ktil"], ktil)
            nc.sync.dma_start(dbg["dbg_v1"], V1)

        # ---------------- phase 4: attention ------------------------------
        wop = tc.alloc_tile_pool(name="wout", bufs=1)
        woh = wop.tile([DH, H, D], f32)  # w_out rows per head at partitions 0:64
        nc.sync.dma_start(R(woh), R(wout_ap.rearrange("(h p) o -> p h o", p=DH)))
        atp = tc.alloc_tile_pool(name="attnT", bufs=1)
        attnT = [atp.tile([DH, N], f32, tag=f"attnT{h}", name=f"attnT{h}") for h in range(H)]
        IH = N // 2  # i-half
        def proj_half(psB, st5, half):
            # output projection for the token range of one i-half
            for t in range(half * NT // 2, (half + 1) * NT // 2):
                pso = psB.tile([P, D], f32, tag="acc", name=f"pso{t}")
                for h in range(H):
                    nc.tensor.matmul(
                        pso,
                        lhsT=R(attnT[h][:, t * P : (t + 1) * P]),
                        rhs=R(woh[:, h, :]),
                        start=(h == 0),
                        stop=(h == H - 1),
                    )
                osb = st5.tile([P, D], f32, tag="osb")
                nc.scalar.copy(osb, pso)
                nc.sync.dma_start(out_ap[t * P : (t + 1) * P, :], osb)

        with tc.tile_pool(name="psS", bufs=2, space="PSUM") as psS, \
             tc.tile_pool(name="psB", bufs=4, space="PSUM") as psB, \
             tc.tile_pool(name="ep", bufs=4) as ep, \
             tc.tile_pool(name="st4", bufs=6) as st4, \
             tc.tile_pool(name="st5", bufs=3) as st5, \
             tc.tile_pool(name="bc4", bufs=4) as bc4:
            for half in range(2):
                ioff = half * IH
                for pr in range(4):
                    h0, h1 = 2 * pr, 2 * pr + 1
                    acc = [
                        psB.tile([DH + 1, 512], f32, tag="acc", name=f"acc{half}_{pr}_{i}") for i in range(4)
                    ]  # [h0 i0, h0 i1, h1 i0, h1 i1]
                    prevE = None
                    for j in range(NT):
                        pss = [psS.tile([P, IH], f32, tag="S", name=f"pss{half}_{pr}_{j}_{i}") for i in range(2)]
                        for hh in range(2):
                            lo = hh * DH
                            for ii in range(2):
                                nc.tensor.matmul(
                                    pss[hh][:, ii * 512 : (ii + 1) * 512],
                                    lhsT=R(ktil[lo : lo + DH, pr, j * P : (j + 1) * P]),
                                    rhs=R(qhat[
                                        lo : lo + DH,
                                        pr,
                                        ioff + ii * 512 : ioff + (ii + 1) * 512,
                                    ]),
                                    start=True,
                                    stop=True,
                                )
                        curE = []
                        for hh, h in ((0, h0), (1, h1)):
                            E = ep.tile([P, IH], bf16, tag="E", name=f"E{half}_{pr}_{j}_{hh}")
                            nc.scalar.activation(
                                E, pss[hh], AF.Exp, scale=SCALE
                            )
                            if dbg is not None and half == 0 and pr == 0 and hh == 0 and j == 0:
                                nc.sync.dma_start(dbg["dbg_E"], E)
                            curE.append(E)
                        if prevE is not None:
                            jp = j - 1
                            for hh, h in ((0, h0), (1, h1)):
                                for ii in range(2):
                                    nc.tensor.matmul(
                                        acc[2 * hh + ii],
                                        lhsT=V1[:, jp, h * (DH + 1) : (h + 1) * (DH + 1)],
                                        rhs=prevE[hh][:, ii * 512 : (ii + 1) * 512],
                                        start=(jp == 0),
                                        stop=False,
                                    )
                        prevE = curE
                    jp = NT - 1
                    for hh, h in ((0, h0), (1, h1)):
                        for ii in range(2):
                            nc.tensor.matmul(
                                acc[2 * hh + ii],
                                lhsT=V1[:, jp, h * (DH + 1) : (h + 1) * (DH + 1)],
                                rhs=prevE[hh][:, ii * 512 : (ii + 1) * 512],
                                start=False,
                                stop=True,
                            )
                    # normalise: row 64 of acc is the softmax denominator.
                    # Stage the raw accumulator to SBUF with one copy so the
                    # PSUM slot frees immediately; the reciprocal/broadcast
                    # chain then runs off the critical path.
                    for hh, h in ((0, h0), (1, h1)):
                        for ii in range(2):
                            a = acc[2 * hh + ii]
                            stg = st4.tile([DH + 1, 512], f32, tag="rec")
                            nc.vector.tensor_copy(stg, a)
                            rd = drec.tile([512], f32, tag="recd")
                            nc.sync.dma_start(
                                rd[None, :], stg[DH : DH + 1, :]
                            )
                            recb = bc4.tile([DH, 512], f32, tag="recb")
                            nc.sync.dma_start(
                                recb,
                                rd[None, :].to_broadcast([DH, 512]),
                            )
                            nc.vector.reciprocal_approx_fast(out=recb, in_=recb)
                            if dbg is not None and half == 0 and pr == 0 and hh == 0 and ii == 0:
                                nc.sync.dma_start(dbg["dbg_rec"], stg)
                                nc.sync.dma_start(dbg["dbg_recb"], recb)
                            nc.vector.tensor_tensor(
                                R(attnT[h][:, ioff + ii * 512 : ioff + (ii + 1) * 512]),
                                stg[0:DH, :],
                                recb,
                                OP.mult,
                            )

                proj_half(psB, st5, half)

            if dbg is not None:
                nc.sync.dma_start(dbg["dbg_attnT0"], attnT[0])

        for pool in (atp, drec, dscr, wop, vp, qkp, const):
            pool.release()


def _build_nc():
    import concourse.mybir as mybir
    from concourse import bacc

    f32 = mybir.dt.float32
    nc = bacc.Bacc("TRN2", target_bir_lowering=False, debug=False)
    x = nc.dram_tensor("x", [N, D], f32, kind="ExternalInput")
    g = nc.dram_tensor("g", [D], f32, kind="ExternalInput")
    w_qkv = nc.dram_tensor("w_qkv", [D, 3 * D], f32, kind="ExternalInput")
    q_scale = nc.dram_tensor("q_scale", [DH], f32, kind="ExternalInput")
    k_scale = nc.dram_tensor("k_scale", [DH], f32, kind="ExternalInput")
    w_out = nc.dram_tensor("w_out", [D, D], f32, kind="ExternalInput")
    out = nc.dram_tensor("out", [N, D], f32, kind="ExternalOutput")
    build_attention(
        nc, out[:], x[:], g[:], w_qkv[:], q_scale[:], k_scale[:], w_out[:]
    )
    nc.finalize()
    return nc


def _bench_spmd(nc, in_maps, n_cores, iters=48, warmup=8):
    """Steady-state device-time estimate: replicate run_bass_via_pjrt's
    jit, pre-stage inputs + donated zero buffers, time K pipelined calls
    and report the per-iteration slope.  NOTE: on axon-tunneled setups
    each call carries ~2.2-2.4 ms of dispatch overhead that is NOT
    pipelined away; the printed number includes it."""
    import time

    import jax
    import numpy as np_
    from jax.sharding import Mesh, PartitionSpec
    from jax.experimental.shard_map import shard_map

    from concourse import bass2jax
    from concourse import mybir

    bass2jax.install_neuronx_cc_hook()
    partition_name = nc.partition_id_tensor.name if nc.partition_id_tensor else None
    in_names, out_names, out_avals, zero_outs = [], [], [], []
    for alloc in nc.m.functions[0].allocations:
        if not isinstance(alloc, mybir.MemoryLocationSet):
            continue
        name = alloc.memorylocations[0].name
        if alloc.kind == "ExternalInput":
            if name != partition_name:
                in_names.append(name)
        elif alloc.kind == "ExternalOutput":
            shape = tuple(alloc.tensor_shape)
            dt = mybir.dt.np(alloc.dtype)
            out_names.append(name)
            out_avals.append(jax.core.ShapedArray(shape, dt))
            zero_outs.append(np_.zeros(shape, dt))
    n_params = len(in_names)
    n_outs = len(out_avals)
    in_names = in_names + out_names
    if partition_name is not None:
        in_names.append(partition_name)
    donate = tuple(range(n_params, n_params + n_outs))

    def _body(*args):
        operands = list(args)
        if partition_name is not None:
            operands.append(bass2jax.partition_id_tensor())
        outs = bass2jax._bass_exec_p.bind(
            *operands,
            out_avals=tuple(out_avals),
            in_names=tuple(in_names),
            out_names=tuple(out_names),
            lowering_input_output_aliases=(),
            sim_require_finite=True,
            sim_require_nnan=True,
            nc=nc,
        )
        return tuple(outs)

    devices = jax.devices()[:n_cores]
    mesh = Mesh(np_.asarray(devices), ("core",))
    sharded = jax.jit(
        shard_map(
            _body,
            mesh=mesh,
            in_specs=(PartitionSpec("core"),) * (n_params + n_outs),
            out_specs=(PartitionSpec("core"),) * len(out_names),
            check_rep=False,
        ),
        donate_argnums=donate,
        keep_unused=True,
    )
    per_core = [[np_.asarray(m[name]) for name in in_names[:n_params]] for m in in_maps]
    concat_in = [
        np_.concatenate([per_core[c][i] for c in range(n_cores)], axis=0)
        for i in range(n_params)
    ]
    sh = jax.sharding.NamedSharding(mesh, PartitionSpec("core"))
    dev_in = [jax.device_put(a, sh) for a in concat_in]

    def zeros_set():
        return [
            jax.device_put(np_.zeros((n_cores * z.shape[0], *z.shape[1:]), z.dtype), sh)
            for z in zero_outs
        ]

    total = warmup + iters
    zsets = [zeros_set() for _ in range(total)]
    for z in zsets:
        jax.block_until_ready(z)
    outs = []
    for i in range(warmup):
        outs = sharded(*dev_in, *zsets[i])
    jax.block_until_ready(outs)
    t0 = time.perf_counter()
    for i in range(warmup, total):
        outs = sharded(*dev_in, *zsets[i])
    jax.block_until_ready(outs)
    t1 = time.perf_counter()
    per_iter_ns = (t1 - t0) / iters * 1e9
    return per_iter_ns


def kernel(x, g, w_qkv, q_scale, k_scale, w_out):
    from concourse.bass_utils import run_bass_kernel_spmd

    nc = _build_nc()
    x = np.ascontiguousarray(np.asarray(x, dtype=np.float32))
    shared = {
        "g": np.asarray(g, np.float32),
        "w_qkv": np.ascontiguousarray(np.asarray(w_qkv, np.float32)),
        "q_scale": np.asarray(q_scale, np.float32),
        "k_scale": np.asarray(k_scale, np.float32),
        "w_out": np.ascontiguousarray(np.asarray(w_out, np.float32)),
    }
    in_maps = [{"x": x[i], **shared} for i in range(NCORES)]
    trace = bool(int(os.environ.get("KERNEL_TRACE", "0")))
    try:
        res = run_bass_kernel_spmd(
            nc, in_maps, core_ids=list(range(NCORES)), trace=trace
        )
    except ModuleNotFoundError:
        # NTFF profile hook unavailable in this container; run untraced.
        res = run_bass_kernel_spmd(
            nc, in_maps, core_ids=list(range(NCORES)), trace=False
        )
    if res.exec_time_ns is not None:
        print(f"HW exec time: {res.exec_time_ns} ns")
    elif int(os.environ.get("KERNEL_BENCH", "0")):
        per_iter = _bench_spmd(nc, in_maps, NCORES)
        print(f"HW exec time: {per_iter:.0f} ns")
    out = np.stack([res.results[i]["out"] for i in range(NCORES)], axis=0)
    return out.astype(np.float32)

